# revision 14
# baseline (speedup 1.0000x reference)
"""Multi-head attention block (QKV proj + SDPA + merge-scramble + fc +
residual + LayerNorm) on 8 Trainium2 NeuronCores.

Sharding: data-parallel over the flattened batch dim (b*n = 32 sequences),
4 sequences per core. Each core runs an identical Bass program on its shard.

Per-sequence math (t = d = e = 512, H = 8 heads, dk = dv = 64):
  Q = qf @ w_q.T ; K = kf @ w_k.T ; V = vf @ w_v.T
  S_h = (Q_h K_h^T) / 8 ;  A_h = softmax(S_h) ;  O_h = A_h V_h
  x = merge_heads(O)            # [t, e]
  x = x.T (the reference's transpose+view scramble; legal since t == e)
  y = LN(x @ w_fc.T + qf) * gamma + beta

Measured-engine-model design (probe-calibrated):
  - bf16 matmul datapath (inputs/weights cast on host), fp32 PSUM,
    fp32 residual + LayerNorm.
  - S^T = K_h Q_h^T with tk on partitions; dk=64 contraction lets two
    heads run as concurrent PE row-tiles; each tk-chunk pair lands in a
    2-bank PSUM tile drained by ONE Act exp over 1024 elements (Act exp
    cost is ~fixed-overhead + stream, so bigger is better).
  - Softmax denominators come from a 32-wide ones block appended to V:
    AV output rows 64..95 are 32 replicated sums rows, drained with one
    multi-row copy (1-row engine ops measured pathological ~3.4us).
    The sums tiles sA/sB are fully written by those copies (no memset).
  - x^T -> x via PE transposes (DMA transpose XBAR measured 2.5us/tile);
    softmax normalization is fused into the transpose drain.
  - 1/sqrt(var+eps) = exp(-0.5*ln(var+eps)) on Act: Ln/Exp/Copy share an
    activation table set, avoiding the ~1.3us table swap Sqrt would cost.
  - Engine budget per seq (est): PE ~26.5us, Act ~23, DVE ~21, Pool ~10.
  - Emission is interleaved at S^T-pair granularity with a filler queue
    (projections of seq s+1, tail of seq s-1) so the in-order PE stream
    never waits on the Act exp chain.
"""

from collections import deque

import numpy as np
import ml_dtypes

import concourse.bacc as bacc
import concourse.mybir as mybir
import concourse.tile as tile
from concourse.bass_utils import run_bass_kernel_spmd
from concourse.masks import make_identity

F32 = mybir.dt.float32
BF16 = mybir.dt.bfloat16
AF = mybir.ActivationFunctionType
OP = mybir.AluOpType

N_CORES = 8
S = 4          # sequences per core
T = 512        # sequence length
D = 512        # model dim (= e = n_head * d_k)
NH = 8         # heads
DV = 64        # head dim
NSUM = 32      # replicated ones columns for softmax sums
C = 4          # 128-row chunks per 512 dim
P = 128
EPS = 1e-6

_PROGRAM_CACHE = {}


def _build_program(apply_affine: bool, loop_iters: int = 1):
    nc = bacc.Bacc()

    qT = nc.declare_dram_parameter("qT", [S, D, T], BF16, isOutput=False)
    kT = nc.declare_dram_parameter("kT", [S, D, T], BF16, isOutput=False)
    vT = nc.declare_dram_parameter("vT", [S, D, T], BF16, isOutput=False)
    qn = nc.declare_dram_parameter("qn", [S, T, D], F32, isOutput=False)
    wq = nc.declare_dram_parameter("wq", [D, D], BF16, isOutput=False)  # w_q.T
    wk = nc.declare_dram_parameter("wk", [D, D], BF16, isOutput=False)  # w_k.T
    wv = nc.declare_dram_parameter("wv", [D, D], BF16, isOutput=False)  # w_v.T
    wfc = nc.declare_dram_parameter("wfc", [D, D], BF16, isOutput=False)  # w_fc.T
    if apply_affine:
        gmb = nc.declare_dram_parameter("gmb", [P, D], F32, isOutput=False)
        btb = nc.declare_dram_parameter("btb", [P, D], F32, isOutput=False)
    out = nc.declare_dram_parameter("out", [S, T, D], F32, isOutput=True)

    with tile.TileContext(nc) as tc:
        with (
            tc.tile_pool(name="const", bufs=1) as cst,
            tc.tile_pool(name="inp", bufs=2) as inp,
            tc.tile_pool(name="proj", bufs=2) as proj,
            tc.tile_pool(name="expp", bufs=2) as expp,
            tc.tile_pool(name="xp", bufs=2) as xp,
            tc.tile_pool(name="small", bufs=2) as small,
            tc.tile_pool(name="pssc", bufs=2, space="PSUM") as pssc,
            tc.tile_pool(name="psmm", bufs=2, space="PSUM") as psmm,
            tc.tile_pool(name="psav", bufs=2, space="PSUM") as psav,
        ):
            wq_sb = cst.tile([P, C, D], BF16, tag="wq")
            wk_sb = cst.tile([P, C, D], BF16, tag="wk")
            wv_sb = cst.tile([P, C, D], BF16, tag="wv")
            wfc_sb = cst.tile([P, C, D], BF16, tag="wfc")
            ident = cst.tile([P, P], F32, tag="ident")
            make_identity(nc, ident[:])
            eps_sb = cst.tile([P, 1], F32, tag="eps")
            nc.vector.memset(eps_sb[:], EPS)
            ones_c = cst.tile([P, NSUM], BF16, tag="ones")
            nc.vector.memset(ones_c[:], 1.0)
            if apply_affine:
                gm_sb = cst.tile([P, D], F32, tag="gmb")
                bt_sb = cst.tile([P, D], F32, tag="btb")
                nc.sync.dma_start(gm_sb[:], gmb[:])
                nc.sync.dma_start(bt_sb[:], btb[:])

            def load_weights():
                # weight DMAs ride the Act HW queue so they overlap the
                # qkv input loads on the SP queue during the prologue
                for w_sb, w in ((wq_sb, wq), (wk_sb, wk),
                                (wv_sb, wv), (wfc_sb, wfc)):
                    for h2 in range(2):
                        nc.scalar.dma_start(
                            w_sb[:, 2 * h2:2 * h2 + 2, :],
                            w.rearrange("(c p) e -> p c e", p=P)[
                                :, 2 * h2:2 * h2 + 2, :],
                        )

            def load_group(s):
                st = {}
                st["qT"] = inp.tile([P, C, T], BF16, tag="qT", name="qT_sb")
                st["kT"] = inp.tile([P, C, T], BF16, tag="kT", name="kT_sb")
                st["vT"] = inp.tile([P, C, T], BF16, tag="vT", name="vT_sb")
                for sb, dr in ((st["qT"], qT), (st["kT"], kT), (st["vT"], vT)):
                    for h2 in range(2):
                        nc.sync.dma_start(
                            sb[:, 2 * h2:2 * h2 + 2, :],
                            dr[s].rearrange("(c p) t -> p c t", p=P)[
                                :, 2 * h2:2 * h2 + 2, :],
                        )
                return st

            def load_qn(s, st):
                st["qn"] = inp.tile([P, C, D], F32, tag="qn", name="qn_sb")
                nc.gpsimd.dma_start(
                    st["qn"][:],
                    qn[s].rearrange("(c p) d -> p c d", p=P)[:],
                )

            def alloc_proj(st, write_ones):
                st["QT"] = proj.tile([P, C, T], BF16, tag="QT", name="QT_sb")
                st["KT"] = proj.tile([P, C, T], BF16, tag="KT", name="KT_sb")
                st["V"] = proj.tile([P, C, NH, DV + NSUM], BF16, tag="V",
                                    name="V_sb")
                if write_ones:
                    # the V-proj copies only touch [0:DV], so the ones block
                    # needs writing once per ring slot (s=0 and s=1)
                    nc.vector.tensor_copy(
                        st["V"][:, :, :, DV:DV + NSUM],
                        ones_c[:, None, None, :].to_broadcast(
                            (P, C, NH, NSUM)),
                    )

            def pq_group(st, ec):
                ps = psmm.tile([P, T], F32, tag="mm", name="ps_q")
                for dc in range(C):
                    nc.tensor.matmul(
                        ps[:],
                        lhsT=wq_sb[:, dc, ec * P:(ec + 1) * P],
                        rhs=st["qT"][:, dc, :],
                        start=(dc == 0), stop=(dc == C - 1),
                    )
                nc.vector.tensor_copy(st["QT"][:, ec, :], ps[:])

            def pk_group(st, ec):
                ps = psmm.tile([P, T], F32, tag="mm", name="ps_k")
                for dc in range(C):
                    nc.tensor.matmul(
                        ps[:],
                        lhsT=wk_sb[:, dc, ec * P:(ec + 1) * P],
                        rhs=st["kT"][:, dc, :],
                        start=(dc == 0), stop=(dc == C - 1),
                    )
                nc.vector.tensor_copy(st["KT"][:, ec, :], ps[:])

            def pv_group(st, tc_):
                ps = psmm.tile([P, T], F32, tag="mm", name="ps_v")
                for dc in range(C):
                    nc.tensor.matmul(
                        ps[:],
                        lhsT=st["vT"][:, dc, tc_ * P:(tc_ + 1) * P],
                        rhs=wv_sb[:, dc, :],
                        start=(dc == 0), stop=(dc == C - 1),
                    )
                nc.scalar.copy(
                    st["V"][:, tc_, :, 0:DV],
                    ps.rearrange("p (h v) -> p h v", h=NH),
                )

            def alloc_attn(st):
                st["xT"] = xp.tile([P, C, T], F32, tag="xT", name="xT_sb")
                st["sA"] = small.tile([P, T], F32, tag="sA", name="sA_sb")
                st["sB"] = small.tile([P, T], F32, tag="sB", name="sB_sb")

            def as_pair(st, hp, tkc):
                # S^T for head-pair hp, tk-chunk tkc: two 64-row PE tiles run
                # concurrently; the pair lands in a 2-bank PSUM tile drained
                # by a single exp over 1024 elements.
                if tkc == 0:
                    st[f"exp{hp}"] = expp.tile(
                        [P, 2, C, T], BF16, tag="exp", name=f"exp{hp}")
                exp_t = st[f"exp{hp}"]
                sc = pssc.tile([P, 2, T], F32, tag="sc", name="sc")
                for sub in range(2):
                    nc.tensor.matmul(
                        sc[:, sub, :],
                        lhsT=st["KT"][sub * DV:(sub + 1) * DV, hp,
                                      tkc * P:(tkc + 1) * P],
                        rhs=st["QT"][sub * DV:(sub + 1) * DV, hp, :],
                        start=True, stop=True,
                    )
                nc.scalar.activation(
                    exp_t[:, :, tkc, :], sc[:, :, :], AF.Exp, scale=0.125,
                )

            def av_group(st, hp):
                # O^T = V_aug^T A^T; 32 ones columns give 32 replicated
                # softmax-sums rows (rows 64..95), drained as one multi-row
                # copy into sA/sB rows {0,32,64,96}.
                exp_t = st[f"exp{hp}"]
                for sub in range(2):
                    h = 2 * hp + sub
                    pav = psav.tile([DV + NSUM, T], F32, tag="av", name="pav")
                    for tkc in range(C):
                        nc.tensor.matmul(
                            pav[:],
                            lhsT=st["V"][:, tkc, h, :],
                            rhs=exp_t[:, sub, tkc, :],
                            start=(tkc == 0), stop=(tkc == C - 1),
                        )
                    nc.vector.tensor_copy(
                        st["xT"][sub * DV:(sub + 1) * DV, hp, :], pav[0:DV, :]
                    )
                    s_t = st["sA"] if h < 4 else st["sB"]
                    nc.vector.tensor_copy(
                        s_t[32 * (h % 4):32 * (h % 4) + 32, :],
                        pav[DV:DV + NSUM, :],
                    )

            def tr_group(st):
                # transpose the sums tiles -> R[tq, head] = 1/sum
                st["R"] = small.tile([P, C, NH], F32, tag="R", name="R_sb")
                for c in range(C):
                    trp = psmm.tile([P, T], F32, tag="mm", name="trp")
                    for i, s_t in enumerate((st["sA"], st["sB"])):
                        nc.tensor.transpose(
                            trp[:, i * P:(i + 1) * P],
                            s_t[:, c * P:(c + 1) * P],
                            ident[:],
                        )
                    nc.vector.reciprocal(st["R"][:, c, 0:4], trp[:, 0:97:32])
                    nc.vector.reciprocal(st["R"][:, c, 4:8],
                                         trp[:, P:P + 97:32])

            def tx_group(st, c):
                # x^T -> x on the PE, normalize fused into the PSUM drain
                if c == 0:
                    st["xn"] = xp.tile([P, C, T], BF16, tag="xn", name="xn_sb")
                ptr = psmm.tile([P, T], F32, tag="mm", name="ptr")
                for ec in range(C):
                    nc.tensor.transpose(
                        ptr[:, ec * P:(ec + 1) * P],
                        st["xT"][:, ec, c * P:(c + 1) * P],
                        ident[:],
                    )
                nc.vector.tensor_tensor(
                    st["xn"][:, c, :].rearrange("p (h v) -> p h v", h=NH),
                    ptr.rearrange("p (h v) -> p h v", h=NH),
                    st["R"][:, c, :, None].to_broadcast((P, NH, DV)),
                    OP.mult,
                )

            def f_group(st, ac):
                # fc (contracting over the *time* index, thanks to the
                # reference's transpose-view scramble) + residual + stats
                if ac == 0:
                    st["y"] = small.tile([P, C, D], F32, tag="y", name="y_sb")
                    st["st2"] = small.tile([P, C, 2], F32, tag="st2",
                                           name="st2_sb")
                psy = psmm.tile([P, T], F32, tag="mm", name="psy")
                for cc in range(C):
                    nc.tensor.matmul(
                        psy[:],
                        lhsT=st["xn"][:, cc, ac * P:(ac + 1) * P],
                        rhs=wfc_sb[:, cc, :],
                        start=(cc == 0), stop=(cc == C - 1),
                    )
                nc.vector.tensor_tensor(
                    st["y"][:, ac, :], psy[:], st["qn"][:, ac, :], OP.add)
                st6 = small.tile([P, 6], F32, tag="st6", name="st6")
                nc.vector.bn_stats(st6[:], st["y"][:, ac, :])
                nc.vector.bn_aggr(st["st2"][:, ac, :], st6[:])

            def n_group(s, st):
                # rinv = exp(-0.5 * ln(var + eps)); Ln/Exp/Copy share an Act
                # table set so no table reload against the softmax exps.
                lnv = small.tile([P, C], F32, tag="lnv", name="lnv")
                rinv = small.tile([P, C], F32, tag="rinv", name="rinv")
                nc.scalar.activation(
                    lnv[:], st["st2"][:, :, 1], AF.Ln, bias=eps_sb[:])
                nc.scalar.activation(rinv[:], lnv[:], AF.Exp, scale=-0.5)
                for ac in range(C):
                    nc.gpsimd.tensor_scalar(
                        st["y"][:, ac, :], st["y"][:, ac, :],
                        st["st2"][:, ac, 0:1], rinv[:, ac:ac + 1],
                        OP.subtract, OP.mult,
                    )
                    if apply_affine:
                        nc.gpsimd.tensor_tensor(
                            st["y"][:, ac, :], st["y"][:, ac, :], gm_sb[:],
                            OP.mult)
                        nc.gpsimd.tensor_tensor(
                            st["y"][:, ac, :], st["y"][:, ac, :], bt_sb[:],
                            OP.add)
                    nc.sync.dma_start(
                        out[s, ac * P:(ac + 1) * P, :], st["y"][:, ac, :])

            def emit_all():
                sts = {}
                load_weights()
                sts[0] = load_group(0)
                load_qn(0, sts[0])
                sts[1] = load_group(1)
                # prologue: project seq 0 while loads land
                alloc_proj(sts[0], True)
                for ec in range(C):
                    pq_group(sts[0], ec)
                for ec in range(C):
                    pk_group(sts[0], ec)
                for tc_ in range(C):
                    pv_group(sts[0], tc_)

                for s in range(S):
                    st = sts[s]
                    prv = sts[s - 1] if s > 0 else None
                    nxt = sts[s + 1] if s + 1 < S else None
                    alloc_attn(st)
                    if nxt is not None:
                        alloc_proj(nxt, s == 0)
                        load_qn(s + 1, nxt)

                    # filler queue: tail of s-1 first (frees buffers), then
                    # projections of s+1.  Emitted one group per S^T pair so
                    # the PE stream always has exp-independent work.
                    fillers = deque()
                    if prv is not None:
                        fillers.append(lambda p=prv: tr_group(p))
                        for c in range(C):
                            fillers.append(lambda p=prv, c=c: tx_group(p, c))
                        for ac in range(C):
                            fillers.append(lambda p=prv, a=ac: f_group(p, a))
                        fillers.append(lambda p=prv, ss=s - 1: n_group(ss, p))
                    if nxt is not None:
                        for ec in range(C):
                            fillers.append(lambda n=nxt, e=ec: pq_group(n, e))
                        for ec in range(C):
                            fillers.append(lambda n=nxt, e=ec: pk_group(n, e))
                        for tc_ in range(C):
                            fillers.append(lambda n=nxt, t=tc_: pv_group(n, t))

                    def fill(k=1):
                        for _ in range(k):
                            if fillers:
                                fillers.popleft()()

                    for hp in range(4):
                        for tkc in range(C):
                            as_pair(st, hp, tkc)
                            fill()
                        if hp >= 1:
                            av_group(st, hp - 1)
                            fill()
                    fill(2)
                    av_group(st, 3)
                    while fillers:
                        fillers.popleft()()
                    if s + 2 < S:
                        sts[s + 2] = load_group(s + 2)

                st = sts[S - 1]
                tr_group(st)
                for c in range(C):
                    tx_group(st, c)
                for ac in range(C):
                    f_group(st, ac)
                n_group(S - 1, st)

            if loop_iters == 1:
                emit_all()
            else:
                with tc.For_i(0, loop_iters, 1):
                    emit_all()

    nc.finalize()
    return nc


def _get_program(apply_affine: bool, loop_iters: int = 1):
    key = (apply_affine, loop_iters)
    if key not in _PROGRAM_CACHE:
        _PROGRAM_CACHE[key] = _build_program(apply_affine, loop_iters)
    return _PROGRAM_CACHE[key]


def _bf16(a):
    return np.ascontiguousarray(a.astype(ml_dtypes.bfloat16))


def kernel(q, k, v, w_q, w_k, w_v, w_fc, ln_gamma, ln_beta, _res_holder=None):
    q = np.asarray(q, dtype=np.float32)
    k = np.asarray(k, dtype=np.float32)
    v = np.asarray(v, dtype=np.float32)
    w_q = np.asarray(w_q, dtype=np.float32)
    w_k = np.asarray(w_k, dtype=np.float32)
    w_v = np.asarray(w_v, dtype=np.float32)
    w_fc = np.asarray(w_fc, dtype=np.float32)
    ln_gamma = np.asarray(ln_gamma, dtype=np.float32)
    ln_beta = np.asarray(ln_beta, dtype=np.float32)

    b, n, t, d = q.shape
    B = b * n
    assert (b, n, t, d) == (8, 4, T, D), q.shape
    qf = q.reshape(B, t, d)
    kf = k.reshape(B, t, d)
    vf = v.reshape(B, t, d)

    apply_affine = not (
        np.all(ln_gamma == 1.0) and np.all(ln_beta == 0.0)
    )
    nc = _get_program(apply_affine)

    wq_t = _bf16(w_q.T)
    wk_t = _bf16(w_k.T)
    wv_t = _bf16(w_v.T)
    wfc_t = _bf16(w_fc.T)

    in_maps = []
    for c in range(N_CORES):
        sl = slice(S * c, S * (c + 1))
        m = {
            "qT": _bf16(qf[sl].transpose(0, 2, 1)),
            "kT": _bf16(kf[sl].transpose(0, 2, 1)),
            "vT": _bf16(vf[sl].transpose(0, 2, 1)),
            "qn": np.ascontiguousarray(qf[sl]),
            "wq": wq_t, "wk": wk_t, "wv": wv_t, "wfc": wfc_t,
        }
        if apply_affine:
            m["gmb"] = np.ascontiguousarray(
                np.broadcast_to(ln_gamma, (P, D)).astype(np.float32)
            )
            m["btb"] = np.ascontiguousarray(
                np.broadcast_to(ln_beta, (P, D)).astype(np.float32)
            )
        in_maps.append(m)

    res = run_bass_kernel_spmd(nc, in_maps, list(range(N_CORES)))
    if _res_holder is not None:
        _res_holder.append(res)
    full = np.concatenate([res.results[c]["out"] for c in range(N_CORES)], axis=0)
    return full.reshape(b, n, t, d).astype(np.float32)
